# revision 11
# baseline (speedup 1.0000x reference)
"""Trainium2 Bass kernel for nn_ChemistryAwareDecoder.

Reference computation (per edge e = (s, d)):
    sp = z[s] * z[d]                       # [128]
    cp = chem[s] * chem[d]                 # [768]
    score_s = relu(sp @ sw1 + sb1) @ sw2 + sb2
    score_c = relu(cp @ cw1 + cb1) @ cw2 + cb2
    score_m = relu(concat(sp, cp) @ mw1 + mb1) @ mw2 + mb2
    t = w0*score_s + w1*score_c + w2*score_m
    bv = mask[s] * mask[d]
    out = bv > 0.5 ? t : score_s

Strategy: data-parallel over edges across 8 NeuronCores. Each core holds a
replicated "combined table" [N, 897] = [z | chem | mask] in DRAM and scores
E/8 edges. Per 512-edge block:
  - 2 indirect-DMA gathers (src rows, dst rows) -> [128, 4*897] SBUF
  - DVE elementwise product (includes mask*mask at col 896)
  - PE transposes product into [feat, edge] layout -> matmuls for 3 MLPs
  - blend on [1, 512] score rows, DMA out
"""

import os
import numpy as np

N_NODES = 100000
E_TOTAL = 200000
SD = 128
CD = 768
F = SD + CD            # 896
FT = F + 1             # 897 table row: z | chem | mask
NCORES = 8
BLK = 512              # edges per block
EPC = E_TOTAL // NCORES            # 25000 edges per core
NBLK = (EPC + BLK - 1) // BLK      # 49
EPAD = NBLK * BLK                  # 25088

LAST_EXEC_NS = None
BF16 = os.environ.get("KERNEL_BF16", "0") == "1"


def _build(n_nodes, nblk, bf16=False):
    import concourse.bass as bass
    import concourse.tile as tile
    from concourse import bacc, mybir
    from concourse.masks import make_identity

    F32 = mybir.dt.float32
    I32 = mybir.dt.int32
    DT = mybir.dt.bfloat16 if bf16 else F32
    AF = mybir.ActivationFunctionType
    OP = mybir.AluOpType

    nc = bacc.Bacc()

    table_d = nc.declare_dram_parameter("table", [n_nodes, FT], DT, isOutput=False)
    eidx_d = nc.declare_dram_parameter("eidx", [128, nblk * 8], I32, isOutput=False)
    sw1_d = nc.declare_dram_parameter("sw1", [128, 64], DT, isOutput=False)
    cw1a_d = nc.declare_dram_parameter("cw1a", [128, 6 * 128], DT, isOutput=False)
    cw1b_d = nc.declare_dram_parameter("cw1b", [128, 6 * 64], DT, isOutput=False)
    mw1p_d = nc.declare_dram_parameter("mw1p", [128, 7 * 128], DT, isOutput=False)
    b1_d = nc.declare_dram_parameter("b1pack", [384], F32, isOutput=False)
    w2_d = nc.declare_dram_parameter("w2pack", [450], DT, isOutput=False)
    out_d = nc.declare_dram_parameter("out", [nblk, BLK], F32, isOutput=True)

    with tile.TileContext(nc) as tc:
        with (
            tc.tile_pool(name="const", bufs=1) as cpool,
            tc.tile_pool(name="gather", bufs=2) as gpool,
            tc.tile_pool(name="prod", bufs=2) as ppool,
            tc.tile_pool(name="prodT", bufs=2) as tpool,
            tc.tile_pool(name="hid", bufs=2) as hpool,
            tc.tile_pool(name="blend", bufs=2) as bpool,
            tc.tile_pool(name="ptr", bufs=2, space="PSUM") as trpool,
            tc.tile_pool(name="ph", bufs=1, space="PSUM") as phpool,
            tc.tile_pool(name="ps", bufs=1, space="PSUM") as pspool,
        ):
            # ---- constants, loaded once ----
            ident = cpool.tile([128, 128], DT, tag="ident")
            make_identity(nc, ident[:])

            eidx_t = cpool.tile([128, nblk * 8], I32, tag="eidx")
            nc.sync.dma_start(out=eidx_t[:], in_=eidx_d[:])

            sw1_t = cpool.tile([128, 64], DT, tag="sw1")
            cw1a_t = cpool.tile([128, 6 * 128], DT, tag="cw1a")
            cw1b_t = cpool.tile([128, 6 * 64], DT, tag="cw1b")
            mw1_t = cpool.tile([128, 7 * 128], DT, tag="mw1")
            nc.sync.dma_start(out=sw1_t[:], in_=sw1_d[:])
            nc.sync.dma_start(out=cw1a_t[:], in_=cw1a_d[:])
            nc.sync.dma_start(out=cw1b_t[:], in_=cw1b_d[:])
            nc.sync.dma_start(out=mw1_t[:], in_=mw1p_d[:])

            sb1_t = cpool.tile([64, 1], F32, tag="sb1")
            cb1a_t = cpool.tile([128, 1], F32, tag="cb1a")
            cb1b_t = cpool.tile([64, 1], F32, tag="cb1b")
            mb1_t = cpool.tile([128, 1], F32, tag="mb1")
            nc.sync.dma_start(out=sb1_t[:], in_=b1_d[0:64])
            nc.sync.dma_start(out=cb1a_t[:], in_=b1_d[64:192])
            nc.sync.dma_start(out=cb1b_t[:], in_=b1_d[192:256])
            nc.sync.dma_start(out=mb1_t[:], in_=b1_d[256:384])

            # w2pack layout: s2 [65] | t2st [65] | t2cha [128] | t2chb [64] | t2cb [128]
            s2_t = cpool.tile([65, 1], DT, tag="s2")
            t2st_t = cpool.tile([65, 1], DT, tag="t2st")
            t2cha_t = cpool.tile([128, 1], DT, tag="t2cha")
            t2chb_t = cpool.tile([64, 1], DT, tag="t2chb")
            t2cb_t = cpool.tile([128, 1], DT, tag="t2cb")
            nc.sync.dma_start(out=s2_t[:], in_=w2_d[0:65])
            nc.sync.dma_start(out=t2st_t[:], in_=w2_d[65:130])
            nc.sync.dma_start(out=t2cha_t[:], in_=w2_d[130:258])
            nc.sync.dma_start(out=t2chb_t[:], in_=w2_d[258:322])
            nc.sync.dma_start(out=t2cb_t[:], in_=w2_d[322:450])

            # ---- per-block pipeline ----
            for b in range(nblk):
                # HW indirect DMA consumes ONE index per dest partition row:
                # 8 gathers of [128, 897] (4 src groups + 4 dst groups).
                comb = gpool.tile([128, 8 * FT], DT, tag="comb")
                for j in range(8):
                    nc.gpsimd.indirect_dma_start(
                        out=comb[:, j * FT:(j + 1) * FT],
                        out_offset=None,
                        in_=table_d[:],
                        in_offset=bass.IndirectOffsetOnAxis(
                            ap=eidx_t[:, b * 8 + j:b * 8 + j + 1], axis=0),
                    )

                prod = ppool.tile([128, 4 * FT], DT, tag="prod")
                nc.vector.tensor_tensor(
                    out=prod[:], in0=comb[:, 0:4 * FT], in1=comb[:, 4 * FT:8 * FT],
                    op=OP.mult)

                # transpose product to [feat, edge]: prodT col = j*128 + p
                prodT = tpool.tile([128, 7 * BLK], DT, tag="prodT")
                for k in range(7):
                    for j in range(4):
                        ptr = trpool.tile([128, 128], DT, tag="ptr")
                        nc.tensor.transpose(
                            out=ptr[:],
                            in_=prod[:, j * FT + k * 128: j * FT + (k + 1) * 128],
                            identity=ident[:],
                        )
                        dst_ap = prodT[:, k * BLK + j * 128: k * BLK + (j + 1) * 128]
                        if (k * 4 + j) % 2 == 0:
                            nc.vector.tensor_copy(out=dst_ap, in_=ptr[:])
                        else:
                            nc.scalar.activation(out=dst_ap, in_=ptr[:], func=AF.Copy)

                # first layers
                p_st = phpool.tile([64, BLK], F32, tag="pst")
                nc.tensor.matmul(p_st[:], lhsT=sw1_t[:], rhs=prodT[:, 0:BLK],
                                 start=True, stop=True)
                p_cha = phpool.tile([128, BLK], F32, tag="pcha")
                for k in range(6):
                    nc.tensor.matmul(
                        p_cha[:], lhsT=cw1a_t[:, k * 128:(k + 1) * 128],
                        rhs=prodT[:, (k + 1) * BLK:(k + 2) * BLK],
                        start=(k == 0), stop=(k == 5))
                p_chb = phpool.tile([64, BLK], F32, tag="pchb")
                for k in range(6):
                    nc.tensor.matmul(
                        p_chb[:], lhsT=cw1b_t[:, k * 64:(k + 1) * 64],
                        rhs=prodT[:, (k + 1) * BLK:(k + 2) * BLK],
                        start=(k == 0), stop=(k == 5))
                p_cb = phpool.tile([128, BLK], F32, tag="pcb")
                for k in range(7):
                    nc.tensor.matmul(
                        p_cb[:], lhsT=mw1_t[:, k * 128:(k + 1) * 128],
                        rhs=prodT[:, k * BLK:(k + 1) * BLK],
                        start=(k == 0), stop=(k == 6))

                # hidden activations (relu + bias), plus a ones-row on hid_st
                # (row 64) that carries the score biases through layer 2.
                hid_st = hpool.tile([65, BLK], DT, tag="hst")
                nc.scalar.activation(out=hid_st[0:64, :], in_=p_st[:],
                                     func=AF.Relu, bias=sb1_t[:])
                nc.gpsimd.memset(hid_st[64:65, :], 1.0)
                hid_cha = hpool.tile([128, BLK], DT, tag="hcha")
                nc.scalar.activation(out=hid_cha[:], in_=p_cha[:],
                                     func=AF.Relu, bias=cb1a_t[:])
                hid_chb = hpool.tile([64, BLK], DT, tag="hchb")
                nc.scalar.activation(out=hid_chb[:], in_=p_chb[:],
                                     func=AF.Relu, bias=cb1b_t[:])
                hid_cb = hpool.tile([128, BLK], DT, tag="hcb")
                nc.scalar.activation(out=hid_cb[:], in_=p_cb[:],
                                     func=AF.Relu, bias=mb1_t[:])

                # second layer: t (weighted sum incl. biases) in its own bank;
                # s (structural-only) at row 0 and bv at row 32 of another bank.
                p_t = pspool.tile([1, BLK], F32, tag="pt")
                nc.tensor.matmul(p_t[0:1, :], lhsT=t2st_t[:], rhs=hid_st[:],
                                 start=True, stop=False)
                nc.tensor.matmul(p_t[0:1, :], lhsT=t2cha_t[:], rhs=hid_cha[:],
                                 start=False, stop=False)
                nc.tensor.matmul(p_t[0:1, :], lhsT=t2chb_t[:], rhs=hid_chb[:],
                                 start=False, stop=False)
                nc.tensor.matmul(p_t[0:1, :], lhsT=t2cb_t[:], rhs=hid_cb[:],
                                 start=False, stop=True)

                p_sbv = pspool.tile([128, BLK], F32, tag="psbv")
                nc.tensor.matmul(p_sbv[0:1, :], lhsT=s2_t[:], rhs=hid_st[:],
                                 start=True, stop=True)
                for j in range(4):
                    nc.tensor.matmul(
                        p_sbv[32:33, j * 128:(j + 1) * 128],
                        lhsT=prod[:, j * FT + F: j * FT + FT],
                        rhs=ident[:],
                        start=True, stop=True)

                # blend: out = s + bv * (t - s)
                # (only one PSUM operand allowed per TT op -> evacuate s first)
                s_sb = bpool.tile([1, BLK], F32, tag="ssb")
                nc.scalar.activation(out=s_sb[:], in_=p_sbv[0:1, :], func=AF.Copy)
                d_t = bpool.tile([1, BLK], F32, tag="d")
                nc.vector.tensor_tensor(out=d_t[:], in0=p_t[0:1, :],
                                        in1=s_sb[:], op=OP.subtract)
                m_t = bpool.tile([1, BLK], F32, tag="m")
                nc.vector.tensor_tensor(out=m_t[:], in0=p_sbv[32:33, :],
                                        in1=d_t[:], op=OP.mult)
                o_t = bpool.tile([1, BLK], F32, tag="o")
                nc.vector.tensor_tensor(out=o_t[:], in0=s_sb[:],
                                        in1=m_t[:], op=OP.add)
                nc.sync.dma_start(out=out_d[b:b + 1, :], in_=o_t[:])

    nc.finalize()
    return nc


def _host_prep(z, chemistry, edge, smiles_mask,
               sw1, sb1, sw2, sb2, cw1, cb1, cw2, cb2, mw1, mb1, mw2, mb2,
               path_weights, n_nodes=N_NODES, nblk=NBLK, ncores=NCORES,
               bf16=False):
    """Build the shared (replicated) arrays + per-core index shards."""
    if bf16:
        import ml_dtypes
        wdt = ml_dtypes.bfloat16
    else:
        wdt = np.float32
    z = np.asarray(z, np.float32)
    chemistry = np.asarray(chemistry, np.float32)
    mask = np.asarray(smiles_mask, np.float32).reshape(-1, 1)
    table = np.concatenate([z, chemistry, mask], axis=1).astype(wdt)
    assert table.shape == (n_nodes, FT)

    pw = np.asarray(path_weights, np.float64)
    e = np.exp(pw - pw.max())
    w = (e / e.sum()).astype(np.float64)
    w0, w1, w2 = [float(x) for x in w]

    sw1 = np.asarray(sw1, np.float32)
    cw1 = np.asarray(cw1, np.float32)
    mw1 = np.asarray(mw1, np.float32)
    cw1a = cw1[:, :128].reshape(6, 128, 128).transpose(1, 0, 2).reshape(128, 6 * 128)
    cw1b = cw1[:, 128:].reshape(6, 128, 64).transpose(1, 0, 2).reshape(128, 6 * 64)
    mw1p = mw1.reshape(7, 128, 128).transpose(1, 0, 2).reshape(128, 7 * 128)
    b1pack = np.concatenate([
        np.asarray(sb1, np.float32),
        np.asarray(cb1, np.float32)[:128],
        np.asarray(cb1, np.float32)[128:],
        np.asarray(mb1, np.float32)]).astype(np.float32)

    sw2 = np.asarray(sw2, np.float64).reshape(-1)
    cw2 = np.asarray(cw2, np.float64).reshape(-1)
    mw2 = np.asarray(mw2, np.float64).reshape(-1)
    sb2v = float(np.asarray(sb2, np.float64).reshape(())[()])
    cb2v = float(np.asarray(cb2, np.float64).reshape(())[()])
    mb2v = float(np.asarray(mb2, np.float64).reshape(())[()])
    tb = w0 * sb2v + w1 * cb2v + w2 * mb2v
    s2 = np.concatenate([sw2, [sb2v]])
    t2st = np.concatenate([w0 * sw2, [tb]])
    w2pack = np.concatenate([
        s2, t2st, w1 * cw2[:128], w1 * cw2[128:], w2 * mw2]).astype(np.float32)
    assert w2pack.shape == (450,)

    edge = np.asarray(edge)
    epc = edge.shape[0] // ncores
    epad = nblk * BLK
    shards = []
    for c in range(ncores):
        sh = edge[c * epc:(c + 1) * epc]
        src = np.zeros(epad, np.int32)
        dst = np.zeros(epad, np.int32)
        src[:epc] = sh[:, 0].astype(np.int32)
        dst[:epc] = sh[:, 1].astype(np.int32)
        # device layout: eidx[p, b*8 + j] = src[b*512 + j*128 + p] for j<4,
        #                                    dst[b*512 + (j-4)*128 + p] for j>=4
        sview = src.reshape(nblk, 4, 128)
        dview = dst.reshape(nblk, 4, 128)
        eidx = np.concatenate([sview, dview], axis=1)  # [nblk, 8, 128]
        eidx = eidx.transpose(2, 0, 1).reshape(128, nblk * 8)
        shards.append(np.ascontiguousarray(eidx))

    shared = dict(table=table, sw1=sw1.astype(wdt),
                  cw1a=np.ascontiguousarray(cw1a).astype(wdt),
                  cw1b=np.ascontiguousarray(cw1b).astype(wdt),
                  mw1p=np.ascontiguousarray(mw1p).astype(wdt),
                  b1pack=b1pack, w2pack=w2pack.astype(wdt))
    return shared, shards, epc


_BUILD_CACHE = {}


def kernel(z, chemistry, edge, smiles_mask,
           sw1, sb1, sw2, sb2, cw1, cb1, cw2, cb2, mw1, mb1, mw2, mb2,
           path_weights):
    global LAST_EXEC_NS
    from concourse import bass_utils
    from concourse.bass_utils import run_bass_kernel_spmd

    trace = os.environ.get("KERNEL_TRACE", "0") == "1"
    if trace:
        # No artifact bucket in this container; keep the NTFF trace local.
        bass_utils.upload_artifacts = lambda tmpdir: tmpdir

    shared, shards, epc = _host_prep(
        z, chemistry, edge, smiles_mask, sw1, sb1, sw2, sb2,
        cw1, cb1, cw2, cb2, mw1, mb1, mw2, mb2, path_weights, bf16=BF16)

    key = (N_NODES, NBLK, BF16)
    if key not in _BUILD_CACHE:
        _BUILD_CACHE[key] = _build(N_NODES, NBLK, bf16=BF16)
    nc = _BUILD_CACHE[key]

    in_maps = []
    for c in range(NCORES):
        m = dict(shared)
        m["eidx"] = shards[c]
        in_maps.append(m)

    tmpdir = os.environ.get("KERNEL_TRACE_DIR") or None
    res = run_bass_kernel_spmd(nc, in_maps, core_ids=list(range(NCORES)),
                               trace=trace, tmpdir=tmpdir)
    if trace:
        LAST_EXEC_NS = res.exec_time_ns

    outs = [r["out"].reshape(-1)[:epc] for r in res.results]
    return np.concatenate(outs).astype(np.float32)


# revision 12
# speedup vs baseline: 1.1277x; 1.1277x over previous
"""Trainium2 Bass kernel for nn_ChemistryAwareDecoder.

Reference computation (per edge e = (s, d)):
    sp = z[s] * z[d]                       # [128]
    cp = chem[s] * chem[d]                 # [768]
    score_s = relu(sp @ sw1 + sb1) @ sw2 + sb2
    score_c = relu(cp @ cw1 + cb1) @ cw2 + cb2
    score_m = relu(concat(sp, cp) @ mw1 + mb1) @ mw2 + mb2
    t = w0*score_s + w1*score_c + w2*score_m
    bv = mask[s] * mask[d]
    out = bv > 0.5 ? t : score_s

Strategy: data-parallel over edges across 8 NeuronCores. Each core holds a
replicated "combined table" [N, 897] = [z | chem | mask] in DRAM and scores
E/8 edges. Per 512-edge block:
  - 2 indirect-DMA gathers (src rows, dst rows) -> [128, 4*897] SBUF
  - DVE elementwise product (includes mask*mask at col 896)
  - PE transposes product into [feat, edge] layout -> matmuls for 3 MLPs
  - blend on [1, 512] score rows, DMA out
"""

import os
import numpy as np

N_NODES = 100000
E_TOTAL = 200000
SD = 128
CD = 768
F = SD + CD            # 896
FT = F + 1             # 897 table row: z | chem | mask
NCORES = 8
BLK = 512              # edges per block
EPC = E_TOTAL // NCORES            # 25000 edges per core
NBLK = (EPC + BLK - 1) // BLK      # 49
EPAD = NBLK * BLK                  # 25088

LAST_EXEC_NS = None
BF16 = os.environ.get("KERNEL_BF16", "0") == "1"


def _build(n_nodes, nblk, bf16=False):
    import concourse.bass as bass
    import concourse.tile as tile
    from concourse import bacc, mybir
    from concourse.masks import make_identity

    F32 = mybir.dt.float32
    I32 = mybir.dt.int32
    DT = mybir.dt.bfloat16 if bf16 else F32
    AF = mybir.ActivationFunctionType
    OP = mybir.AluOpType

    nc = bacc.Bacc()

    table_d = nc.declare_dram_parameter("table", [n_nodes, FT], DT, isOutput=False)
    eidx_d = nc.declare_dram_parameter("eidx", [128, nblk * 8], I32, isOutput=False)
    sw1_d = nc.declare_dram_parameter("sw1", [128, 64], DT, isOutput=False)
    cw1a_d = nc.declare_dram_parameter("cw1a", [128, 6 * 128], DT, isOutput=False)
    cw1b_d = nc.declare_dram_parameter("cw1b", [128, 6 * 64], DT, isOutput=False)
    mw1p_d = nc.declare_dram_parameter("mw1p", [128, 7 * 128], DT, isOutput=False)
    b1_d = nc.declare_dram_parameter("b1pack", [384], F32, isOutput=False)
    w2_d = nc.declare_dram_parameter("w2pack", [450], DT, isOutput=False)
    out_d = nc.declare_dram_parameter("out", [nblk, BLK], F32, isOutput=True)

    with tile.TileContext(nc) as tc:
        with (
            tc.tile_pool(name="const", bufs=1) as cpool,
            tc.tile_pool(name="gather", bufs=2) as gpool,
            tc.tile_pool(name="prod", bufs=2) as ppool,
            tc.tile_pool(name="prodT", bufs=2) as tpool,
            tc.tile_pool(name="hid", bufs=2) as hpool,
            tc.tile_pool(name="blend", bufs=2) as bpool,
            tc.tile_pool(name="ptr", bufs=2, space="PSUM") as trpool,
            tc.tile_pool(name="ph", bufs=1, space="PSUM") as phpool,
            tc.tile_pool(name="ps", bufs=1, space="PSUM") as pspool,
        ):
            # ---- constants, loaded once ----
            ident = cpool.tile([128, 128], DT, tag="ident")
            make_identity(nc, ident[:])

            eidx_t = cpool.tile([128, nblk * 8], I32, tag="eidx")
            nc.sync.dma_start(out=eidx_t[:], in_=eidx_d[:])

            sw1_t = cpool.tile([128, 64], DT, tag="sw1")
            cw1a_t = cpool.tile([128, 6 * 128], DT, tag="cw1a")
            cw1b_t = cpool.tile([128, 6 * 64], DT, tag="cw1b")
            mw1_t = cpool.tile([128, 7 * 128], DT, tag="mw1")
            nc.sync.dma_start(out=sw1_t[:], in_=sw1_d[:])
            nc.sync.dma_start(out=cw1a_t[:], in_=cw1a_d[:])
            nc.sync.dma_start(out=cw1b_t[:], in_=cw1b_d[:])
            nc.sync.dma_start(out=mw1_t[:], in_=mw1p_d[:])

            sb1_t = cpool.tile([64, 1], F32, tag="sb1")
            cb1a_t = cpool.tile([128, 1], F32, tag="cb1a")
            cb1b_t = cpool.tile([64, 1], F32, tag="cb1b")
            mb1_t = cpool.tile([128, 1], F32, tag="mb1")
            nc.sync.dma_start(out=sb1_t[:], in_=b1_d[0:64])
            nc.sync.dma_start(out=cb1a_t[:], in_=b1_d[64:192])
            nc.sync.dma_start(out=cb1b_t[:], in_=b1_d[192:256])
            nc.sync.dma_start(out=mb1_t[:], in_=b1_d[256:384])

            # w2pack layout: s2 [65] | t2st [65] | t2cha [128] | t2chb [64] | t2cb [128]
            s2_t = cpool.tile([65, 1], DT, tag="s2")
            t2st_t = cpool.tile([65, 1], DT, tag="t2st")
            t2cha_t = cpool.tile([128, 1], DT, tag="t2cha")
            t2chb_t = cpool.tile([64, 1], DT, tag="t2chb")
            t2cb_t = cpool.tile([128, 1], DT, tag="t2cb")
            nc.sync.dma_start(out=s2_t[:], in_=w2_d[0:65])
            nc.sync.dma_start(out=t2st_t[:], in_=w2_d[65:130])
            nc.sync.dma_start(out=t2cha_t[:], in_=w2_d[130:258])
            nc.sync.dma_start(out=t2chb_t[:], in_=w2_d[258:322])
            nc.sync.dma_start(out=t2cb_t[:], in_=w2_d[322:450])

            # ---- per-block pipeline ----
            for b in range(nblk):
                # HW indirect DMA consumes ONE index per dest partition row:
                # 8 gathers of [128, 897] (4 src groups + 4 dst groups).
                comb = gpool.tile([128, 8 * FT], DT, tag="comb")
                for j in range(8):
                    nc.gpsimd.indirect_dma_start(
                        out=comb[:, j * FT:(j + 1) * FT],
                        out_offset=None,
                        in_=table_d[:],
                        in_offset=bass.IndirectOffsetOnAxis(
                            ap=eidx_t[:, b * 8 + j:b * 8 + j + 1], axis=0),
                    )

                prod = ppool.tile([128, 4 * FT], DT, tag="prod")
                nc.vector.tensor_tensor(
                    out=prod[:], in0=comb[:, 0:4 * FT], in1=comb[:, 4 * FT:8 * FT],
                    op=OP.mult)

                # transpose product to [feat, edge]: prodT col = j*128 + p.
                # 4 transposes pack into one PSUM tile -> one wide evacuation.
                prodT = tpool.tile([128, 7 * BLK], DT, tag="prodT")
                for k in range(7):
                    ptr4 = trpool.tile([128, BLK], DT, tag="ptr")
                    for j in range(4):
                        nc.tensor.transpose(
                            out=ptr4[:, j * 128:(j + 1) * 128],
                            in_=prod[:, j * FT + k * 128: j * FT + (k + 1) * 128],
                            identity=ident[:],
                        )
                    dst_ap = prodT[:, k * BLK:(k + 1) * BLK]
                    if k % 2 == 0:
                        nc.vector.tensor_copy(out=dst_ap, in_=ptr4[:])
                    else:
                        nc.scalar.activation(out=dst_ap, in_=ptr4[:], func=AF.Copy)

                # first layers
                p_st = phpool.tile([64, BLK], F32, tag="pst")
                nc.tensor.matmul(p_st[:], lhsT=sw1_t[:], rhs=prodT[:, 0:BLK],
                                 start=True, stop=True)
                p_cha = phpool.tile([128, BLK], F32, tag="pcha")
                for k in range(6):
                    nc.tensor.matmul(
                        p_cha[:], lhsT=cw1a_t[:, k * 128:(k + 1) * 128],
                        rhs=prodT[:, (k + 1) * BLK:(k + 2) * BLK],
                        start=(k == 0), stop=(k == 5))
                p_chb = phpool.tile([64, BLK], F32, tag="pchb")
                for k in range(6):
                    nc.tensor.matmul(
                        p_chb[:], lhsT=cw1b_t[:, k * 64:(k + 1) * 64],
                        rhs=prodT[:, (k + 1) * BLK:(k + 2) * BLK],
                        start=(k == 0), stop=(k == 5))
                p_cb = phpool.tile([128, BLK], F32, tag="pcb")
                for k in range(7):
                    nc.tensor.matmul(
                        p_cb[:], lhsT=mw1_t[:, k * 128:(k + 1) * 128],
                        rhs=prodT[:, k * BLK:(k + 1) * BLK],
                        start=(k == 0), stop=(k == 6))

                # hidden activations (relu + bias), plus a ones-row on hid_st
                # (row 64) that carries the score biases through layer 2.
                hid_st = hpool.tile([65, BLK], DT, tag="hst")
                nc.scalar.activation(out=hid_st[0:64, :], in_=p_st[:],
                                     func=AF.Relu, bias=sb1_t[:])
                nc.gpsimd.memset(hid_st[64:65, :], 1.0)
                hid_cha = hpool.tile([128, BLK], DT, tag="hcha")
                nc.scalar.activation(out=hid_cha[:], in_=p_cha[:],
                                     func=AF.Relu, bias=cb1a_t[:])
                hid_chb = hpool.tile([64, BLK], DT, tag="hchb")
                nc.scalar.activation(out=hid_chb[:], in_=p_chb[:],
                                     func=AF.Relu, bias=cb1b_t[:])
                hid_cb = hpool.tile([128, BLK], DT, tag="hcb")
                nc.scalar.activation(out=hid_cb[:], in_=p_cb[:],
                                     func=AF.Relu, bias=mb1_t[:])

                # second layer: t (weighted sum incl. biases) in its own bank;
                # s (structural-only) at row 0 and bv at row 32 of another bank.
                p_t = pspool.tile([1, BLK], F32, tag="pt")
                nc.tensor.matmul(p_t[0:1, :], lhsT=t2st_t[:], rhs=hid_st[:],
                                 start=True, stop=False)
                nc.tensor.matmul(p_t[0:1, :], lhsT=t2cha_t[:], rhs=hid_cha[:],
                                 start=False, stop=False)
                nc.tensor.matmul(p_t[0:1, :], lhsT=t2chb_t[:], rhs=hid_chb[:],
                                 start=False, stop=False)
                nc.tensor.matmul(p_t[0:1, :], lhsT=t2cb_t[:], rhs=hid_cb[:],
                                 start=False, stop=True)

                p_sbv = pspool.tile([128, BLK], F32, tag="psbv")
                nc.tensor.matmul(p_sbv[0:1, :], lhsT=s2_t[:], rhs=hid_st[:],
                                 start=True, stop=True)
                for j in range(4):
                    nc.tensor.matmul(
                        p_sbv[32:33, j * 128:(j + 1) * 128],
                        lhsT=prod[:, j * FT + F: j * FT + FT],
                        rhs=ident[:],
                        start=True, stop=True)

                # blend: out = s + bv * (t - s)
                # (only one PSUM operand allowed per TT op -> evacuate s first)
                s_sb = bpool.tile([1, BLK], F32, tag="ssb")
                nc.scalar.activation(out=s_sb[:], in_=p_sbv[0:1, :], func=AF.Copy)
                d_t = bpool.tile([1, BLK], F32, tag="d")
                nc.vector.tensor_tensor(out=d_t[:], in0=p_t[0:1, :],
                                        in1=s_sb[:], op=OP.subtract)
                m_t = bpool.tile([1, BLK], F32, tag="m")
                nc.vector.tensor_tensor(out=m_t[:], in0=p_sbv[32:33, :],
                                        in1=d_t[:], op=OP.mult)
                o_t = bpool.tile([1, BLK], F32, tag="o")
                nc.vector.tensor_tensor(out=o_t[:], in0=s_sb[:],
                                        in1=m_t[:], op=OP.add)
                nc.sync.dma_start(out=out_d[b:b + 1, :], in_=o_t[:])

    nc.finalize()
    return nc


def _host_prep(z, chemistry, edge, smiles_mask,
               sw1, sb1, sw2, sb2, cw1, cb1, cw2, cb2, mw1, mb1, mw2, mb2,
               path_weights, n_nodes=N_NODES, nblk=NBLK, ncores=NCORES,
               bf16=False):
    """Build the shared (replicated) arrays + per-core index shards."""
    if bf16:
        import ml_dtypes
        wdt = ml_dtypes.bfloat16
    else:
        wdt = np.float32
    z = np.asarray(z, np.float32)
    chemistry = np.asarray(chemistry, np.float32)
    mask = np.asarray(smiles_mask, np.float32).reshape(-1, 1)
    table = np.concatenate([z, chemistry, mask], axis=1).astype(wdt)
    assert table.shape == (n_nodes, FT)

    pw = np.asarray(path_weights, np.float64)
    e = np.exp(pw - pw.max())
    w = (e / e.sum()).astype(np.float64)
    w0, w1, w2 = [float(x) for x in w]

    sw1 = np.asarray(sw1, np.float32)
    cw1 = np.asarray(cw1, np.float32)
    mw1 = np.asarray(mw1, np.float32)
    cw1a = cw1[:, :128].reshape(6, 128, 128).transpose(1, 0, 2).reshape(128, 6 * 128)
    cw1b = cw1[:, 128:].reshape(6, 128, 64).transpose(1, 0, 2).reshape(128, 6 * 64)
    mw1p = mw1.reshape(7, 128, 128).transpose(1, 0, 2).reshape(128, 7 * 128)
    b1pack = np.concatenate([
        np.asarray(sb1, np.float32),
        np.asarray(cb1, np.float32)[:128],
        np.asarray(cb1, np.float32)[128:],
        np.asarray(mb1, np.float32)]).astype(np.float32)

    sw2 = np.asarray(sw2, np.float64).reshape(-1)
    cw2 = np.asarray(cw2, np.float64).reshape(-1)
    mw2 = np.asarray(mw2, np.float64).reshape(-1)
    sb2v = float(np.asarray(sb2, np.float64).reshape(())[()])
    cb2v = float(np.asarray(cb2, np.float64).reshape(())[()])
    mb2v = float(np.asarray(mb2, np.float64).reshape(())[()])
    tb = w0 * sb2v + w1 * cb2v + w2 * mb2v
    s2 = np.concatenate([sw2, [sb2v]])
    t2st = np.concatenate([w0 * sw2, [tb]])
    w2pack = np.concatenate([
        s2, t2st, w1 * cw2[:128], w1 * cw2[128:], w2 * mw2]).astype(np.float32)
    assert w2pack.shape == (450,)

    edge = np.asarray(edge)
    epc = edge.shape[0] // ncores
    epad = nblk * BLK
    shards = []
    for c in range(ncores):
        sh = edge[c * epc:(c + 1) * epc]
        src = np.zeros(epad, np.int32)
        dst = np.zeros(epad, np.int32)
        src[:epc] = sh[:, 0].astype(np.int32)
        dst[:epc] = sh[:, 1].astype(np.int32)
        # device layout: eidx[p, b*8 + j] = src[b*512 + j*128 + p] for j<4,
        #                                    dst[b*512 + (j-4)*128 + p] for j>=4
        sview = src.reshape(nblk, 4, 128)
        dview = dst.reshape(nblk, 4, 128)
        eidx = np.concatenate([sview, dview], axis=1)  # [nblk, 8, 128]
        eidx = eidx.transpose(2, 0, 1).reshape(128, nblk * 8)
        shards.append(np.ascontiguousarray(eidx))

    shared = dict(table=table, sw1=sw1.astype(wdt),
                  cw1a=np.ascontiguousarray(cw1a).astype(wdt),
                  cw1b=np.ascontiguousarray(cw1b).astype(wdt),
                  mw1p=np.ascontiguousarray(mw1p).astype(wdt),
                  b1pack=b1pack, w2pack=w2pack.astype(wdt))
    return shared, shards, epc


_BUILD_CACHE = {}


def kernel(z, chemistry, edge, smiles_mask,
           sw1, sb1, sw2, sb2, cw1, cb1, cw2, cb2, mw1, mb1, mw2, mb2,
           path_weights):
    global LAST_EXEC_NS
    from concourse import bass_utils
    from concourse.bass_utils import run_bass_kernel_spmd

    trace = os.environ.get("KERNEL_TRACE", "0") == "1"
    if trace:
        # No artifact bucket in this container; keep the NTFF trace local.
        bass_utils.upload_artifacts = lambda tmpdir: tmpdir

    shared, shards, epc = _host_prep(
        z, chemistry, edge, smiles_mask, sw1, sb1, sw2, sb2,
        cw1, cb1, cw2, cb2, mw1, mb1, mw2, mb2, path_weights, bf16=BF16)

    key = (N_NODES, NBLK, BF16)
    if key not in _BUILD_CACHE:
        _BUILD_CACHE[key] = _build(N_NODES, NBLK, bf16=BF16)
    nc = _BUILD_CACHE[key]

    in_maps = []
    for c in range(NCORES):
        m = dict(shared)
        m["eidx"] = shards[c]
        in_maps.append(m)

    tmpdir = os.environ.get("KERNEL_TRACE_DIR") or None
    res = run_bass_kernel_spmd(nc, in_maps, core_ids=list(range(NCORES)),
                               trace=trace, tmpdir=tmpdir)
    if trace:
        LAST_EXEC_NS = res.exec_time_ns

    outs = [r["out"].reshape(-1)[:epc] for r in res.results]
    return np.concatenate(outs).astype(np.float32)


# revision 14
# speedup vs baseline: 1.1371x; 1.0083x over previous
"""Trainium2 Bass kernel for nn_ChemistryAwareDecoder.

Reference computation (per edge e = (s, d)):
    sp = z[s] * z[d]                       # [128]
    cp = chem[s] * chem[d]                 # [768]
    score_s = relu(sp @ sw1 + sb1) @ sw2 + sb2
    score_c = relu(cp @ cw1 + cb1) @ cw2 + cb2
    score_m = relu(concat(sp, cp) @ mw1 + mb1) @ mw2 + mb2
    t = w0*score_s + w1*score_c + w2*score_m
    bv = mask[s] * mask[d]
    out = bv > 0.5 ? t : score_s

Strategy: data-parallel over edges across 8 NeuronCores. Each core holds a
replicated "combined table" [N, 897] = [z | chem | mask] in DRAM and scores
E/8 edges. Per 512-edge block:
  - 2 indirect-DMA gathers (src rows, dst rows) -> [128, 4*897] SBUF
  - DVE elementwise product (includes mask*mask at col 896)
  - PE transposes product into [feat, edge] layout -> matmuls for 3 MLPs
  - blend on [1, 512] score rows, DMA out
"""

import os
import numpy as np

N_NODES = 100000
E_TOTAL = 200000
SD = 128
CD = 768
F = SD + CD            # 896
FT = F + 1             # 897 table row: z | chem | mask
NCORES = 8
BLK = 512              # edges per block
EPC = E_TOTAL // NCORES            # 25000 edges per core
NBLK = (EPC + BLK - 1) // BLK      # 49
EPAD = NBLK * BLK                  # 25088

LAST_EXEC_NS = None
BF16 = os.environ.get("KERNEL_BF16", "0") == "1"


def _build(n_nodes, nblk, bf16=False):
    import concourse.bass as bass
    import concourse.tile as tile
    from concourse import bacc, mybir
    from concourse.masks import make_identity

    F32 = mybir.dt.float32
    I32 = mybir.dt.int32
    DT = mybir.dt.bfloat16 if bf16 else F32
    AF = mybir.ActivationFunctionType
    OP = mybir.AluOpType

    nc = bacc.Bacc()

    table_d = nc.declare_dram_parameter("table", [n_nodes, FT], DT, isOutput=False)
    eidx_d = nc.declare_dram_parameter("eidx", [128, nblk * 8], I32, isOutput=False)
    sw1_d = nc.declare_dram_parameter("sw1", [128, 64], DT, isOutput=False)
    cw1a_d = nc.declare_dram_parameter("cw1a", [128, 6 * 128], DT, isOutput=False)
    cw1b_d = nc.declare_dram_parameter("cw1b", [128, 6 * 64], DT, isOutput=False)
    mw1p_d = nc.declare_dram_parameter("mw1p", [128, 7 * 128], DT, isOutput=False)
    b1_d = nc.declare_dram_parameter("b1pack", [384], F32, isOutput=False)
    w2_d = nc.declare_dram_parameter("w2pack", [450], DT, isOutput=False)
    out_d = nc.declare_dram_parameter("out", [nblk, BLK], F32, isOutput=True)

    with tile.TileContext(nc) as tc:
        with (
            tc.tile_pool(name="const", bufs=1) as cpool,
            tc.tile_pool(name="gather", bufs=3) as gpool,
            tc.tile_pool(name="prod", bufs=3) as ppool,
            tc.tile_pool(name="prodT", bufs=2) as tpool,
            tc.tile_pool(name="hid", bufs=2) as hpool,
            tc.tile_pool(name="blend", bufs=2) as bpool,
            tc.tile_pool(name="ptr", bufs=2, space="PSUM") as trpool,
            tc.tile_pool(name="ph", bufs=1, space="PSUM") as phpool,
            tc.tile_pool(name="ps", bufs=1, space="PSUM") as pspool,
        ):
            # ---- constants, loaded once ----
            ident = cpool.tile([128, 128], DT, tag="ident")
            make_identity(nc, ident[:])

            eidx_t = cpool.tile([128, nblk * 8], I32, tag="eidx")
            nc.sync.dma_start(out=eidx_t[:], in_=eidx_d[:])

            sw1_t = cpool.tile([128, 64], DT, tag="sw1")
            cw1a_t = cpool.tile([128, 6 * 128], DT, tag="cw1a")
            cw1b_t = cpool.tile([128, 6 * 64], DT, tag="cw1b")
            mw1_t = cpool.tile([128, 7 * 128], DT, tag="mw1")
            nc.sync.dma_start(out=sw1_t[:], in_=sw1_d[:])
            nc.sync.dma_start(out=cw1a_t[:], in_=cw1a_d[:])
            nc.sync.dma_start(out=cw1b_t[:], in_=cw1b_d[:])
            nc.sync.dma_start(out=mw1_t[:], in_=mw1p_d[:])

            sb1_t = cpool.tile([64, 1], F32, tag="sb1")
            cb1a_t = cpool.tile([128, 1], F32, tag="cb1a")
            cb1b_t = cpool.tile([64, 1], F32, tag="cb1b")
            mb1_t = cpool.tile([128, 1], F32, tag="mb1")
            nc.sync.dma_start(out=sb1_t[:], in_=b1_d[0:64])
            nc.sync.dma_start(out=cb1a_t[:], in_=b1_d[64:192])
            nc.sync.dma_start(out=cb1b_t[:], in_=b1_d[192:256])
            nc.sync.dma_start(out=mb1_t[:], in_=b1_d[256:384])

            # w2pack layout: s2 [65] | t2st [65] | t2cha [128] | t2chb [64] | t2cb [128]
            s2_t = cpool.tile([65, 1], DT, tag="s2")
            t2st_t = cpool.tile([65, 1], DT, tag="t2st")
            t2cha_t = cpool.tile([128, 1], DT, tag="t2cha")
            t2chb_t = cpool.tile([64, 1], DT, tag="t2chb")
            t2cb_t = cpool.tile([128, 1], DT, tag="t2cb")
            nc.sync.dma_start(out=s2_t[:], in_=w2_d[0:65])
            nc.sync.dma_start(out=t2st_t[:], in_=w2_d[65:130])
            nc.sync.dma_start(out=t2cha_t[:], in_=w2_d[130:258])
            nc.sync.dma_start(out=t2chb_t[:], in_=w2_d[258:322])
            nc.sync.dma_start(out=t2cb_t[:], in_=w2_d[322:450])

            # persistent double-buffered structural-hidden tiles; row 64 is a
            # constant ones-row (carries the layer-2 biases), written once.
            hst_bufs = [cpool.tile([65, BLK], DT, name=f"hst{i}", tag=f"hst{i}")
                        for i in range(2)]
            for t in hst_bufs:
                nc.gpsimd.memset(t[64:65, :], 1.0)

            # ---- per-block pipeline ----
            for b in range(nblk):
                # HW indirect DMA consumes ONE index per dest partition row:
                # 8 gathers of [128, 897] (4 src groups + 4 dst groups).
                comb = gpool.tile([128, 8 * FT], DT, tag="comb")
                for j in range(8):
                    nc.gpsimd.indirect_dma_start(
                        out=comb[:, j * FT:(j + 1) * FT],
                        out_offset=None,
                        in_=table_d[:],
                        in_offset=bass.IndirectOffsetOnAxis(
                            ap=eidx_t[:, b * 8 + j:b * 8 + j + 1], axis=0),
                    )

                prod = ppool.tile([128, 4 * FT], DT, tag="prod")
                nc.vector.tensor_tensor(
                    out=prod[:], in0=comb[:, 0:4 * FT], in1=comb[:, 4 * FT:8 * FT],
                    op=OP.mult)

                # transpose product to [feat, edge]: prodT col = j*128 + p.
                # 4 transposes pack into one PSUM tile -> one wide evacuation.
                prodT = tpool.tile([128, 7 * BLK], DT, tag="prodT")
                for k in range(7):
                    ptr4 = trpool.tile([128, BLK], DT, tag="ptr")
                    for j in range(4):
                        nc.tensor.transpose(
                            out=ptr4[:, j * 128:(j + 1) * 128],
                            in_=prod[:, j * FT + k * 128: j * FT + (k + 1) * 128],
                            identity=ident[:],
                        )
                    dst_ap = prodT[:, k * BLK:(k + 1) * BLK]
                    if k % 2 == 0:
                        nc.vector.tensor_copy(out=dst_ap, in_=ptr4[:])
                    else:
                        nc.scalar.activation(out=dst_ap, in_=ptr4[:], func=AF.Copy)

                # first layers
                p_st = phpool.tile([64, BLK], F32, tag="pst")
                nc.tensor.matmul(p_st[:], lhsT=sw1_t[:], rhs=prodT[:, 0:BLK],
                                 start=True, stop=True)
                p_cha = phpool.tile([128, BLK], F32, tag="pcha")
                for k in range(6):
                    nc.tensor.matmul(
                        p_cha[:], lhsT=cw1a_t[:, k * 128:(k + 1) * 128],
                        rhs=prodT[:, (k + 1) * BLK:(k + 2) * BLK],
                        start=(k == 0), stop=(k == 5))
                p_chb = phpool.tile([64, BLK], F32, tag="pchb")
                for k in range(6):
                    nc.tensor.matmul(
                        p_chb[:], lhsT=cw1b_t[:, k * 64:(k + 1) * 64],
                        rhs=prodT[:, (k + 1) * BLK:(k + 2) * BLK],
                        start=(k == 0), stop=(k == 5))
                p_cb = phpool.tile([128, BLK], F32, tag="pcb")
                for k in range(7):
                    nc.tensor.matmul(
                        p_cb[:], lhsT=mw1_t[:, k * 128:(k + 1) * 128],
                        rhs=prodT[:, k * BLK:(k + 1) * BLK],
                        start=(k == 0), stop=(k == 6))

                # hidden activations (relu + bias), plus a ones-row on hid_st
                # (row 64) that carries the score biases through layer 2.
                hid_st = hst_bufs[b % 2]
                nc.scalar.activation(out=hid_st[0:64, :], in_=p_st[:],
                                     func=AF.Relu, bias=sb1_t[:])
                hid_cha = hpool.tile([128, BLK], DT, tag="hcha")
                nc.scalar.activation(out=hid_cha[:], in_=p_cha[:],
                                     func=AF.Relu, bias=cb1a_t[:])
                hid_chb = hpool.tile([64, BLK], DT, tag="hchb")
                nc.scalar.activation(out=hid_chb[:], in_=p_chb[:],
                                     func=AF.Relu, bias=cb1b_t[:])
                hid_cb = hpool.tile([128, BLK], DT, tag="hcb")
                nc.scalar.activation(out=hid_cb[:], in_=p_cb[:],
                                     func=AF.Relu, bias=mb1_t[:])

                # second layer: t (weighted sum incl. biases) in its own bank;
                # s (structural-only) at row 0 and bv at row 32 of another bank.
                p_t = pspool.tile([1, BLK], F32, tag="pt")
                nc.tensor.matmul(p_t[0:1, :], lhsT=t2st_t[:], rhs=hid_st[:],
                                 start=True, stop=False)
                nc.tensor.matmul(p_t[0:1, :], lhsT=t2cha_t[:], rhs=hid_cha[:],
                                 start=False, stop=False)
                nc.tensor.matmul(p_t[0:1, :], lhsT=t2chb_t[:], rhs=hid_chb[:],
                                 start=False, stop=False)
                nc.tensor.matmul(p_t[0:1, :], lhsT=t2cb_t[:], rhs=hid_cb[:],
                                 start=False, stop=True)

                p_sbv = pspool.tile([128, BLK], F32, tag="psbv")
                nc.tensor.matmul(p_sbv[0:1, :], lhsT=s2_t[:], rhs=hid_st[:],
                                 start=True, stop=True)
                for j in range(4):
                    nc.tensor.matmul(
                        p_sbv[32:33, j * 128:(j + 1) * 128],
                        lhsT=prod[:, j * FT + F: j * FT + FT],
                        rhs=ident[:],
                        start=True, stop=True)

                # blend: out = s + bv * (t - s)
                # (only one PSUM operand allowed per TT op -> evacuate s first)
                s_sb = bpool.tile([1, BLK], F32, tag="ssb")
                nc.scalar.activation(out=s_sb[:], in_=p_sbv[0:1, :], func=AF.Copy)
                d_t = bpool.tile([1, BLK], F32, tag="d")
                nc.vector.tensor_tensor(out=d_t[:], in0=p_t[0:1, :],
                                        in1=s_sb[:], op=OP.subtract)
                m_t = bpool.tile([1, BLK], F32, tag="m")
                nc.vector.tensor_tensor(out=m_t[:], in0=p_sbv[32:33, :],
                                        in1=d_t[:], op=OP.mult)
                o_t = bpool.tile([1, BLK], F32, tag="o")
                nc.vector.tensor_tensor(out=o_t[:], in0=s_sb[:],
                                        in1=m_t[:], op=OP.add)
                nc.sync.dma_start(out=out_d[b:b + 1, :], in_=o_t[:])

    nc.finalize()
    return nc


def _host_prep(z, chemistry, edge, smiles_mask,
               sw1, sb1, sw2, sb2, cw1, cb1, cw2, cb2, mw1, mb1, mw2, mb2,
               path_weights, n_nodes=N_NODES, nblk=NBLK, ncores=NCORES,
               bf16=False):
    """Build the shared (replicated) arrays + per-core index shards."""
    if bf16:
        import ml_dtypes
        wdt = ml_dtypes.bfloat16
    else:
        wdt = np.float32
    z = np.asarray(z, np.float32)
    chemistry = np.asarray(chemistry, np.float32)
    mask = np.asarray(smiles_mask, np.float32).reshape(-1, 1)
    table = np.concatenate([z, chemistry, mask], axis=1).astype(wdt)
    assert table.shape == (n_nodes, FT)

    pw = np.asarray(path_weights, np.float64)
    e = np.exp(pw - pw.max())
    w = (e / e.sum()).astype(np.float64)
    w0, w1, w2 = [float(x) for x in w]

    sw1 = np.asarray(sw1, np.float32)
    cw1 = np.asarray(cw1, np.float32)
    mw1 = np.asarray(mw1, np.float32)
    cw1a = cw1[:, :128].reshape(6, 128, 128).transpose(1, 0, 2).reshape(128, 6 * 128)
    cw1b = cw1[:, 128:].reshape(6, 128, 64).transpose(1, 0, 2).reshape(128, 6 * 64)
    mw1p = mw1.reshape(7, 128, 128).transpose(1, 0, 2).reshape(128, 7 * 128)
    b1pack = np.concatenate([
        np.asarray(sb1, np.float32),
        np.asarray(cb1, np.float32)[:128],
        np.asarray(cb1, np.float32)[128:],
        np.asarray(mb1, np.float32)]).astype(np.float32)

    sw2 = np.asarray(sw2, np.float64).reshape(-1)
    cw2 = np.asarray(cw2, np.float64).reshape(-1)
    mw2 = np.asarray(mw2, np.float64).reshape(-1)
    sb2v = float(np.asarray(sb2, np.float64).reshape(())[()])
    cb2v = float(np.asarray(cb2, np.float64).reshape(())[()])
    mb2v = float(np.asarray(mb2, np.float64).reshape(())[()])
    tb = w0 * sb2v + w1 * cb2v + w2 * mb2v
    s2 = np.concatenate([sw2, [sb2v]])
    t2st = np.concatenate([w0 * sw2, [tb]])
    w2pack = np.concatenate([
        s2, t2st, w1 * cw2[:128], w1 * cw2[128:], w2 * mw2]).astype(np.float32)
    assert w2pack.shape == (450,)

    edge = np.asarray(edge)
    epc = edge.shape[0] // ncores
    epad = nblk * BLK
    shards = []
    for c in range(ncores):
        sh = edge[c * epc:(c + 1) * epc]
        src = np.zeros(epad, np.int32)
        dst = np.zeros(epad, np.int32)
        src[:epc] = sh[:, 0].astype(np.int32)
        dst[:epc] = sh[:, 1].astype(np.int32)
        # device layout: eidx[p, b*8 + j] = src[b*512 + j*128 + p] for j<4,
        #                                    dst[b*512 + (j-4)*128 + p] for j>=4
        sview = src.reshape(nblk, 4, 128)
        dview = dst.reshape(nblk, 4, 128)
        eidx = np.concatenate([sview, dview], axis=1)  # [nblk, 8, 128]
        eidx = eidx.transpose(2, 0, 1).reshape(128, nblk * 8)
        shards.append(np.ascontiguousarray(eidx))

    shared = dict(table=table, sw1=sw1.astype(wdt),
                  cw1a=np.ascontiguousarray(cw1a).astype(wdt),
                  cw1b=np.ascontiguousarray(cw1b).astype(wdt),
                  mw1p=np.ascontiguousarray(mw1p).astype(wdt),
                  b1pack=b1pack, w2pack=w2pack.astype(wdt))
    return shared, shards, epc


_BUILD_CACHE = {}


def kernel(z, chemistry, edge, smiles_mask,
           sw1, sb1, sw2, sb2, cw1, cb1, cw2, cb2, mw1, mb1, mw2, mb2,
           path_weights):
    global LAST_EXEC_NS
    from concourse import bass_utils
    from concourse.bass_utils import run_bass_kernel_spmd

    trace = os.environ.get("KERNEL_TRACE", "0") == "1"
    if trace:
        # No artifact bucket in this container; keep the NTFF trace local.
        bass_utils.upload_artifacts = lambda tmpdir: tmpdir

    shared, shards, epc = _host_prep(
        z, chemistry, edge, smiles_mask, sw1, sb1, sw2, sb2,
        cw1, cb1, cw2, cb2, mw1, mb1, mw2, mb2, path_weights, bf16=BF16)

    key = (N_NODES, NBLK, BF16)
    if key not in _BUILD_CACHE:
        _BUILD_CACHE[key] = _build(N_NODES, NBLK, bf16=BF16)
    nc = _BUILD_CACHE[key]

    in_maps = []
    for c in range(NCORES):
        m = dict(shared)
        m["eidx"] = shards[c]
        in_maps.append(m)

    tmpdir = os.environ.get("KERNEL_TRACE_DIR") or None
    res = run_bass_kernel_spmd(nc, in_maps, core_ids=list(range(NCORES)),
                               trace=trace, tmpdir=tmpdir)
    if trace:
        LAST_EXEC_NS = res.exec_time_ns

    outs = [r["out"].reshape(-1)[:epc] for r in res.results]
    return np.concatenate(outs).astype(np.float32)


# revision 15
# speedup vs baseline: 1.3481x; 1.1856x over previous
"""Trainium2 Bass kernel for nn_ChemistryAwareDecoder.

Reference computation (per edge e = (s, d)):
    sp = z[s] * z[d]                       # [128]
    cp = chem[s] * chem[d]                 # [768]
    score_s = relu(sp @ sw1 + sb1) @ sw2 + sb2
    score_c = relu(cp @ cw1 + cb1) @ cw2 + cb2
    score_m = relu(concat(sp, cp) @ mw1 + mb1) @ mw2 + mb2
    t = w0*score_s + w1*score_c + w2*score_m
    bv = mask[s] * mask[d]
    out = bv > 0.5 ? t : score_s

Strategy: data-parallel over edges across 8 NeuronCores, bf16 compute.
Each core holds a replicated padded node table [N, 1024] = [z | chem | mask |
0-pad] in DRAM. Edges are sorted by src on the host so each core's src values
fit a 32768-row window (int16 indices), and within a core edges are bucketed
by dst into 4 windows of N/4 rows (int16 again). Per 512-edge block:
  - 2 transposing dma_gathers (src rows, dst rows) -> [128 feat-part, 8, 512]
    SBUF tiles, i.e. the gathered rows arrive already transposed
  - one DVE elementwise product = transposed pair products (mask product
    lands on partition 0 of chunk 7 -> bv row for free)
  - matmuls for the 3 MLPs (first layer contracts feat chunks 0..6),
    second layer includes a ones-row that carries the score biases
  - blend on [1, 512] score rows, DMA out; host unpermutes to edge order
"""

import os
import numpy as np

N_NODES = 100000
E_TOTAL = 200000
SD = 128
CD = 768
F = SD + CD            # 896 real features
ELEM = 1024            # padded table row (bf16 -> 2048B, %256==0)
NCORES = 8
BLK = 512              # edges per block
NBUCK = 4
SRCWIN = 32768

LAST_EXEC_NS = None


def _build(n_nodes, bucket_blocks, srcwin):
    import concourse.bass as bass  # noqa: F401
    import concourse.tile as tile
    from concourse import bacc, mybir

    F32 = mybir.dt.float32
    I16 = mybir.dt.int16
    DT = mybir.dt.bfloat16
    AF = mybir.ActivationFunctionType
    OP = mybir.AluOpType

    dstwin = -(-n_nodes // NBUCK)
    nblk = sum(bucket_blocks)
    bucket_of = [g for g in range(NBUCK) for _ in range(bucket_blocks[g])]

    nc = bacc.Bacc()

    table_d = nc.declare_dram_parameter("table", [n_nodes, ELEM], DT, isOutput=False)
    stable_d = nc.declare_dram_parameter("stable", [srcwin, ELEM], DT, isOutput=False)
    eidx_d = nc.declare_dram_parameter("eidx", [128, nblk * 64], I16, isOutput=False)
    sw1_d = nc.declare_dram_parameter("sw1", [128, 64], DT, isOutput=False)
    cw1a_d = nc.declare_dram_parameter("cw1a", [128, 6 * 128], DT, isOutput=False)
    cw1b_d = nc.declare_dram_parameter("cw1b", [128, 6 * 64], DT, isOutput=False)
    mw1p_d = nc.declare_dram_parameter("mw1p", [128, 7 * 128], DT, isOutput=False)
    b1_d = nc.declare_dram_parameter("b1pack", [384], F32, isOutput=False)
    w2_d = nc.declare_dram_parameter("w2pack", [450], DT, isOutput=False)
    out_d = nc.declare_dram_parameter("out", [nblk, BLK], F32, isOutput=True)

    with tile.TileContext(nc) as tc:
        with (
            tc.tile_pool(name="const", bufs=1) as cpool,
            tc.tile_pool(name="gather", bufs=3) as gpool,
            tc.tile_pool(name="prod", bufs=3) as ppool,
            tc.tile_pool(name="hid", bufs=2) as hpool,
            tc.tile_pool(name="blend", bufs=2) as bpool,
            tc.tile_pool(name="ph", bufs=1, space="PSUM") as phpool,
            tc.tile_pool(name="ps", bufs=2, space="PSUM") as pspool,
        ):
            # ---- constants, loaded once ----
            eidx_t = cpool.tile([128, nblk * 64], I16, tag="eidx")
            nc.sync.dma_start(out=eidx_t[:], in_=eidx_d[:])

            sw1_t = cpool.tile([128, 64], DT, tag="sw1")
            cw1a_t = cpool.tile([128, 6 * 128], DT, tag="cw1a")
            cw1b_t = cpool.tile([128, 6 * 64], DT, tag="cw1b")
            mw1_t = cpool.tile([128, 7 * 128], DT, tag="mw1")
            nc.sync.dma_start(out=sw1_t[:], in_=sw1_d[:])
            nc.sync.dma_start(out=cw1a_t[:], in_=cw1a_d[:])
            nc.sync.dma_start(out=cw1b_t[:], in_=cw1b_d[:])
            nc.sync.dma_start(out=mw1_t[:], in_=mw1p_d[:])

            sb1_t = cpool.tile([64, 1], F32, tag="sb1")
            cb1a_t = cpool.tile([128, 1], F32, tag="cb1a")
            cb1b_t = cpool.tile([64, 1], F32, tag="cb1b")
            mb1_t = cpool.tile([128, 1], F32, tag="mb1")
            nc.sync.dma_start(out=sb1_t[:], in_=b1_d[0:64])
            nc.sync.dma_start(out=cb1a_t[:], in_=b1_d[64:192])
            nc.sync.dma_start(out=cb1b_t[:], in_=b1_d[192:256])
            nc.sync.dma_start(out=mb1_t[:], in_=b1_d[256:384])

            # w2pack layout: s2 [65] | t2st [65] | t2cha [128] | t2chb [64] | t2cb [128]
            s2_t = cpool.tile([65, 1], DT, tag="s2")
            t2st_t = cpool.tile([65, 1], DT, tag="t2st")
            t2cha_t = cpool.tile([128, 1], DT, tag="t2cha")
            t2chb_t = cpool.tile([64, 1], DT, tag="t2chb")
            t2cb_t = cpool.tile([128, 1], DT, tag="t2cb")
            nc.sync.dma_start(out=s2_t[:], in_=w2_d[0:65])
            nc.sync.dma_start(out=t2st_t[:], in_=w2_d[65:130])
            nc.sync.dma_start(out=t2cha_t[:], in_=w2_d[130:258])
            nc.sync.dma_start(out=t2chb_t[:], in_=w2_d[258:322])
            nc.sync.dma_start(out=t2cb_t[:], in_=w2_d[322:450])

            # persistent double-buffered structural-hidden tiles; row 64 is a
            # constant ones-row (carries the layer-2 biases), written once.
            hst_bufs = [cpool.tile([65, BLK], DT, name=f"hst{i}", tag=f"hst{i}")
                        for i in range(2)]
            for t in hst_bufs:
                nc.gpsimd.memset(t[64:65, :], 1.0)

            # ---- per-block pipeline ----
            for b in range(nblk):
                g = bucket_of[b]
                # transposing gathers: out[a, c, i] = table[idx_i, c*128 + a]
                srcT = gpool.tile([128, 8 * BLK], DT, tag="srcT")
                dstT = gpool.tile([128, 8 * BLK], DT, tag="dstT")
                nc.gpsimd.dma_gather(
                    out_ap=srcT[:].rearrange("p (c e) -> p c e", e=BLK),
                    in_ap=stable_d[:],
                    idxs_ap=eidx_t[:, b * 64:b * 64 + 32],
                    num_idxs=BLK, num_idxs_reg=BLK,
                    elem_size=ELEM, transpose=True,
                )
                nc.gpsimd.dma_gather(
                    out_ap=dstT[:].rearrange("p (c e) -> p c e", e=BLK),
                    in_ap=table_d[g * dstwin:(g + 1) * dstwin, :],
                    idxs_ap=eidx_t[:, b * 64 + 32:b * 64 + 64],
                    num_idxs=BLK, num_idxs_reg=BLK,
                    elem_size=ELEM, transpose=True,
                )

                # pair products, already in [feat, edge] layout; chunk 7 row 0
                # is mask_src*mask_dst = bv.
                prodT = ppool.tile([128, 8 * BLK], DT, tag="prodT")
                nc.vector.tensor_tensor(
                    out=prodT[:], in0=srcT[:], in1=dstT[:], op=OP.mult)

                # first layers (contract feat chunks: 0 structural, 1..6 chem)
                p_st = phpool.tile([64, BLK], F32, tag="pst")
                nc.tensor.matmul(p_st[:], lhsT=sw1_t[:], rhs=prodT[:, 0:BLK],
                                 start=True, stop=True)
                p_cha = phpool.tile([128, BLK], F32, tag="pcha")
                for k in range(6):
                    nc.tensor.matmul(
                        p_cha[:], lhsT=cw1a_t[:, k * 128:(k + 1) * 128],
                        rhs=prodT[:, (k + 1) * BLK:(k + 2) * BLK],
                        start=(k == 0), stop=(k == 5))
                p_chb = phpool.tile([64, BLK], F32, tag="pchb")
                for k in range(6):
                    nc.tensor.matmul(
                        p_chb[:], lhsT=cw1b_t[:, k * 64:(k + 1) * 64],
                        rhs=prodT[:, (k + 1) * BLK:(k + 2) * BLK],
                        start=(k == 0), stop=(k == 5))
                p_cb = phpool.tile([128, BLK], F32, tag="pcb")
                for k in range(7):
                    nc.tensor.matmul(
                        p_cb[:], lhsT=mw1_t[:, k * 128:(k + 1) * 128],
                        rhs=prodT[:, k * BLK:(k + 1) * BLK],
                        start=(k == 0), stop=(k == 6))

                # hidden activations (relu + bias)
                hid_st = hst_bufs[b % 2]
                nc.scalar.activation(out=hid_st[0:64, :], in_=p_st[:],
                                     func=AF.Relu, bias=sb1_t[:])
                hid_cha = hpool.tile([128, BLK], DT, tag="hcha")
                nc.scalar.activation(out=hid_cha[:], in_=p_cha[:],
                                     func=AF.Relu, bias=cb1a_t[:])
                hid_chb = hpool.tile([64, BLK], DT, tag="hchb")
                nc.scalar.activation(out=hid_chb[:], in_=p_chb[:],
                                     func=AF.Relu, bias=cb1b_t[:])
                hid_cb = hpool.tile([128, BLK], DT, tag="hcb")
                nc.scalar.activation(out=hid_cb[:], in_=p_cb[:],
                                     func=AF.Relu, bias=mb1_t[:])

                # second layer: t (weighted sum incl. biases via ones-row),
                # s (structural-only score) in separate psum banks.
                p_t = pspool.tile([1, BLK], F32, tag="pt")
                nc.tensor.matmul(p_t[0:1, :], lhsT=t2st_t[:], rhs=hid_st[:],
                                 start=True, stop=False)
                nc.tensor.matmul(p_t[0:1, :], lhsT=t2cha_t[:], rhs=hid_cha[:],
                                 start=False, stop=False)
                nc.tensor.matmul(p_t[0:1, :], lhsT=t2chb_t[:], rhs=hid_chb[:],
                                 start=False, stop=False)
                nc.tensor.matmul(p_t[0:1, :], lhsT=t2cb_t[:], rhs=hid_cb[:],
                                 start=False, stop=True)
                p_s = pspool.tile([1, BLK], F32, tag="psc")
                nc.tensor.matmul(p_s[0:1, :], lhsT=s2_t[:], rhs=hid_st[:],
                                 start=True, stop=True)

                # blend: out = s + bv * (t - s); bv is prodT chunk-7 row 0.
                s_sb = bpool.tile([1, BLK], F32, tag="ssb")
                nc.scalar.activation(out=s_sb[:], in_=p_s[0:1, :], func=AF.Copy)
                d_t = bpool.tile([1, BLK], F32, tag="d")
                nc.vector.tensor_tensor(out=d_t[:], in0=p_t[0:1, :],
                                        in1=s_sb[:], op=OP.subtract)
                m_t = bpool.tile([1, BLK], F32, tag="m")
                nc.vector.tensor_tensor(out=m_t[:], in0=prodT[0:1, 7 * BLK:8 * BLK],
                                        in1=d_t[:], op=OP.mult)
                o_t = bpool.tile([1, BLK], F32, tag="o")
                nc.vector.tensor_tensor(out=o_t[:], in0=s_sb[:],
                                        in1=m_t[:], op=OP.add)
                nc.sync.dma_start(out=out_d[b:b + 1, :], in_=o_t[:])

    nc.finalize()
    return nc


def _host_prep(z, chemistry, edge, smiles_mask,
               sw1, sb1, sw2, sb2, cw1, cb1, cw2, cb2, mw1, mb1, mw2, mb2,
               path_weights, n_nodes=N_NODES, ncores=NCORES):
    """Sort/bucket edges, build the padded bf16 table + per-core shards."""
    import ml_dtypes
    wdt = ml_dtypes.bfloat16

    z = np.asarray(z, np.float32)
    chemistry = np.asarray(chemistry, np.float32)
    mask = np.asarray(smiles_mask, np.float32).reshape(-1)
    table = np.zeros((n_nodes, ELEM), np.float32)
    table[:, :SD] = z
    table[:, SD:F] = chemistry
    table[:, F] = mask
    table = table.astype(wdt)

    srcwin = min(SRCWIN, n_nodes)
    dstwin = -(-n_nodes // NBUCK)
    assert dstwin <= 32767

    pw = np.asarray(path_weights, np.float64)
    e = np.exp(pw - pw.max())
    w = e / e.sum()
    w0, w1, w2 = [float(x) for x in w]

    sw1 = np.asarray(sw1, np.float32)
    cw1 = np.asarray(cw1, np.float32)
    mw1 = np.asarray(mw1, np.float32)
    cw1a = cw1[:, :128].reshape(6, 128, 128).transpose(1, 0, 2).reshape(128, 6 * 128)
    cw1b = cw1[:, 128:].reshape(6, 128, 64).transpose(1, 0, 2).reshape(128, 6 * 64)
    mw1p = mw1.reshape(7, 128, 128).transpose(1, 0, 2).reshape(128, 7 * 128)
    b1pack = np.concatenate([
        np.asarray(sb1, np.float32),
        np.asarray(cb1, np.float32)[:128],
        np.asarray(cb1, np.float32)[128:],
        np.asarray(mb1, np.float32)]).astype(np.float32)

    sw2v = np.asarray(sw2, np.float64).reshape(-1)
    cw2v = np.asarray(cw2, np.float64).reshape(-1)
    mw2v = np.asarray(mw2, np.float64).reshape(-1)
    sb2v = float(np.asarray(sb2, np.float64).reshape(())[()])
    cb2v = float(np.asarray(cb2, np.float64).reshape(())[()])
    mb2v = float(np.asarray(mb2, np.float64).reshape(())[()])
    tb = w0 * sb2v + w1 * cb2v + w2 * mb2v
    w2pack = np.concatenate([
        np.concatenate([sw2v, [sb2v]]),
        np.concatenate([w0 * sw2v, [tb]]),
        w1 * cw2v[:128], w1 * cw2v[128:], w2 * mw2v]).astype(np.float32)
    assert w2pack.shape == (450,)

    edge = np.asarray(edge)
    E = edge.shape[0]
    src_all = edge[:, 0].astype(np.int64)
    dst_all = edge[:, 1].astype(np.int64)
    order = np.argsort(src_all, kind='stable')
    epc = E // ncores

    cores = []
    counts_all = np.zeros((ncores, NBUCK), np.int64)
    for c in range(ncores):
        ids = order[c * epc:(c + 1) * epc]
        s = src_all[ids]
        d = dst_all[ids]
        w0c = max(0, min(int(s.min()), n_nodes - srcwin))
        assert int(s.max()) - w0c < srcwin, "src window overflow"
        g = d // dstwin
        bord = np.argsort(g, kind='stable')
        ids, s, d, g = ids[bord], s[bord], d[bord], g[bord]
        counts_all[c] = np.bincount(g, minlength=NBUCK)
        cores.append((ids, s - w0c, d - g * dstwin, g, w0c))

    bucket_blocks = tuple(int(-(-int(counts_all[:, gg].max()) // BLK))
                          for gg in range(NBUCK))
    bucket_blocks = tuple(max(1, bb) for bb in bucket_blocks)
    nblk = sum(bucket_blocks)

    shards = []
    for c in range(ncores):
        ids, s_rel, d_rel, g, w0c = cores[c]
        src16 = np.zeros(nblk * BLK, np.int16)
        dst16 = np.zeros(nblk * BLK, np.int16)
        perm = np.full(nblk * BLK, -1, np.int64)
        base_blk = 0
        pos = 0
        for gg in range(NBUCK):
            n_g = int(counts_all[c, gg])
            sl = slice(base_blk * BLK, base_blk * BLK + n_g)
            src16[sl] = s_rel[pos:pos + n_g].astype(np.int16)
            dst16[sl] = d_rel[pos:pos + n_g].astype(np.int16)
            perm[sl] = ids[pos:pos + n_g]
            pos += n_g
            base_blk += bucket_blocks[gg]
        # per-block idx wrap: flat pos k -> [k%16, k//16], replicated x8
        ar = np.arange(BLK)
        eidx = np.zeros((16, nblk * 64), np.int16)
        for b in range(nblk):
            sblk = src16[b * BLK:(b + 1) * BLK]
            dblk = dst16[b * BLK:(b + 1) * BLK]
            eidx[ar % 16, b * 64 + ar // 16] = sblk
            eidx[ar % 16, b * 64 + 32 + ar // 16] = dblk
        eidx = np.tile(eidx, (8, 1))
        stable = np.ascontiguousarray(table[w0c:w0c + srcwin])
        shards.append((eidx, stable, perm))

    shared = dict(table=table, sw1=sw1.astype(wdt),
                  cw1a=np.ascontiguousarray(cw1a).astype(wdt),
                  cw1b=np.ascontiguousarray(cw1b).astype(wdt),
                  mw1p=np.ascontiguousarray(mw1p).astype(wdt),
                  b1pack=b1pack, w2pack=w2pack.astype(wdt))
    return shared, shards, bucket_blocks, srcwin, E


_BUILD_CACHE = {}


def kernel(z, chemistry, edge, smiles_mask,
           sw1, sb1, sw2, sb2, cw1, cb1, cw2, cb2, mw1, mb1, mw2, mb2,
           path_weights):
    global LAST_EXEC_NS
    from concourse import bass_utils
    from concourse.bass_utils import run_bass_kernel_spmd

    trace = os.environ.get("KERNEL_TRACE", "0") == "1"
    if trace:
        # No artifact bucket in this container; keep the NTFF trace local.
        bass_utils.upload_artifacts = lambda tmpdir: tmpdir

    shared, shards, bucket_blocks, srcwin, E = _host_prep(
        z, chemistry, edge, smiles_mask, sw1, sb1, sw2, sb2,
        cw1, cb1, cw2, cb2, mw1, mb1, mw2, mb2, path_weights)

    key = (N_NODES, bucket_blocks, srcwin)
    if key not in _BUILD_CACHE:
        _BUILD_CACHE[key] = _build(N_NODES, bucket_blocks, srcwin)
    nc = _BUILD_CACHE[key]

    in_maps = []
    for c in range(NCORES):
        m = dict(shared)
        m["eidx"], m["stable"], _ = shards[c]
        in_maps.append(m)

    tmpdir = os.environ.get("KERNEL_TRACE_DIR") or None
    res = run_bass_kernel_spmd(nc, in_maps, core_ids=list(range(NCORES)),
                               trace=trace, tmpdir=tmpdir)
    if trace:
        LAST_EXEC_NS = res.exec_time_ns

    result = np.zeros(E, np.float32)
    for c in range(NCORES):
        perm = shards[c][2]
        dev = res.results[c]["out"].reshape(-1)
        valid = perm >= 0
        result[perm[valid]] = dev[valid]
    return result


# revision 16
# speedup vs baseline: 1.7701x; 1.3131x over previous
"""Trainium2 Bass kernel for nn_ChemistryAwareDecoder.

Reference computation (per edge e = (s, d)):
    sp = z[s] * z[d]                       # [128]
    cp = chem[s] * chem[d]                 # [768]
    score_s = relu(sp @ sw1 + sb1) @ sw2 + sb2
    score_c = relu(cp @ cw1 + cb1) @ cw2 + cb2
    score_m = relu(concat(sp, cp) @ mw1 + mb1) @ mw2 + mb2
    t = w0*score_s + w1*score_c + w2*score_m
    bv = mask[s] * mask[d]
    out = bv > 0.5 ? t : score_s

Strategy: data-parallel over edges across 8 NeuronCores, bf16 compute.
Each core holds a replicated padded node table [N, 1024] = [z | chem | mask |
0-pad] in DRAM. Edges are sorted by src on the host so each core's src values
fit a 32768-row window (int16 indices), and within a core edges are bucketed
by dst into 4 windows of N/4 rows (int16 again). Per 512-edge block:
  - 2 transposing dma_gathers (src rows, dst rows) -> [128 feat-part, 8, 512]
    SBUF tiles, i.e. the gathered rows arrive already transposed
  - one DVE elementwise product = transposed pair products (mask product
    lands on partition 0 of chunk 7 -> bv row for free)
  - matmuls for the 3 MLPs (first layer contracts feat chunks 0..6),
    second layer includes a ones-row that carries the score biases
  - blend on [1, 512] score rows, DMA out; host unpermutes to edge order
"""

import os
import numpy as np

N_NODES = 100000
E_TOTAL = 200000
SD = 128
CD = 768
F = SD + CD            # 896 real features
ELEM = 1024            # padded table row (bf16 -> 2048B, %256==0)
NCORES = 8
BLK = 512              # edges per block
NBUCK = 4
SRCWIN = 32768

LAST_EXEC_NS = None


def _build(n_nodes, bucket_blocks, srcwin):
    import os
    import concourse.bass as bass  # noqa: F401
    import concourse.tile as tile
    from concourse import bacc, mybir

    F32 = mybir.dt.float32
    I16 = mybir.dt.int16
    DT = mybir.dt.bfloat16
    AF = mybir.ActivationFunctionType
    OP = mybir.AluOpType

    dstwin = -(-n_nodes // NBUCK)
    nblk = sum(bucket_blocks)
    bucket_of = [g for g in range(NBUCK) for _ in range(bucket_blocks[g])]

    nc = bacc.Bacc(num_swdge_queues=int(os.environ.get("KERNEL_NSWQ", "1")))

    table_d = nc.declare_dram_parameter("table", [n_nodes, ELEM], DT, isOutput=False)
    stable_d = nc.declare_dram_parameter("stable", [srcwin, ELEM], DT, isOutput=False)
    eidx_d = nc.declare_dram_parameter("eidx", [128, nblk * 64], I16, isOutput=False)
    sw1_d = nc.declare_dram_parameter("sw1", [128, 64], DT, isOutput=False)
    cw1a_d = nc.declare_dram_parameter("cw1a", [128, 6 * 128], DT, isOutput=False)
    cw1b_d = nc.declare_dram_parameter("cw1b", [128, 6 * 64], DT, isOutput=False)
    mw1p_d = nc.declare_dram_parameter("mw1p", [128, 7 * 128], DT, isOutput=False)
    b1_d = nc.declare_dram_parameter("b1pack", [384], F32, isOutput=False)
    w2_d = nc.declare_dram_parameter("w2pack", [450], DT, isOutput=False)
    out_d = nc.declare_dram_parameter("out", [nblk, BLK], F32, isOutput=True)

    with tile.TileContext(nc) as tc:
        with (
            tc.tile_pool(name="const", bufs=1) as cpool,
            tc.tile_pool(name="gather", bufs=3) as gpool,
            tc.tile_pool(name="prod", bufs=3) as ppool,
            tc.tile_pool(name="hid", bufs=2) as hpool,
            tc.tile_pool(name="blend", bufs=2) as bpool,
            tc.tile_pool(name="ph", bufs=1, space="PSUM") as phpool,
            tc.tile_pool(name="ps", bufs=2, space="PSUM") as pspool,
        ):
            # ---- constants, loaded once ----
            eidx_t = cpool.tile([128, nblk * 64], I16, tag="eidx")
            nc.sync.dma_start(out=eidx_t[:], in_=eidx_d[:])

            sw1_t = cpool.tile([128, 64], DT, tag="sw1")
            cw1a_t = cpool.tile([128, 6 * 128], DT, tag="cw1a")
            cw1b_t = cpool.tile([128, 6 * 64], DT, tag="cw1b")
            mw1_t = cpool.tile([128, 7 * 128], DT, tag="mw1")
            nc.sync.dma_start(out=sw1_t[:], in_=sw1_d[:])
            nc.sync.dma_start(out=cw1a_t[:], in_=cw1a_d[:])
            nc.sync.dma_start(out=cw1b_t[:], in_=cw1b_d[:])
            nc.sync.dma_start(out=mw1_t[:], in_=mw1p_d[:])

            sb1_t = cpool.tile([64, 1], F32, tag="sb1")
            cb1a_t = cpool.tile([128, 1], F32, tag="cb1a")
            cb1b_t = cpool.tile([64, 1], F32, tag="cb1b")
            mb1_t = cpool.tile([128, 1], F32, tag="mb1")
            nc.sync.dma_start(out=sb1_t[:], in_=b1_d[0:64])
            nc.sync.dma_start(out=cb1a_t[:], in_=b1_d[64:192])
            nc.sync.dma_start(out=cb1b_t[:], in_=b1_d[192:256])
            nc.sync.dma_start(out=mb1_t[:], in_=b1_d[256:384])

            # w2pack layout: s2 [65] | t2st [65] | t2cha [128] | t2chb [64] | t2cb [128]
            s2_t = cpool.tile([65, 1], DT, tag="s2")
            t2st_t = cpool.tile([65, 1], DT, tag="t2st")
            t2cha_t = cpool.tile([128, 1], DT, tag="t2cha")
            t2chb_t = cpool.tile([64, 1], DT, tag="t2chb")
            t2cb_t = cpool.tile([128, 1], DT, tag="t2cb")
            nc.sync.dma_start(out=s2_t[:], in_=w2_d[0:65])
            nc.sync.dma_start(out=t2st_t[:], in_=w2_d[65:130])
            nc.sync.dma_start(out=t2cha_t[:], in_=w2_d[130:258])
            nc.sync.dma_start(out=t2chb_t[:], in_=w2_d[258:322])
            nc.sync.dma_start(out=t2cb_t[:], in_=w2_d[322:450])

            # persistent double-buffered structural-hidden tiles; row 64 is a
            # constant ones-row (carries the layer-2 biases), written once.
            hst_bufs = [cpool.tile([65, BLK], DT, name=f"hst{i}", tag=f"hst{i}")
                        for i in range(2)]
            for t in hst_bufs:
                nc.gpsimd.memset(t[64:65, :], 1.0)

            # ---- per-block pipeline ----
            for b in range(nblk):
                g = bucket_of[b]
                # transposing gathers: out[a, c, i] = table[idx_i, c*128 + a]
                srcT = gpool.tile([128, 8 * BLK], DT, tag="srcT")
                dstT = gpool.tile([128, 8 * BLK], DT, tag="dstT")
                nc.gpsimd.dma_gather(
                    out_ap=srcT[:].rearrange("p (c e) -> p c e", e=BLK),
                    in_ap=stable_d[:],
                    idxs_ap=eidx_t[:, b * 64:b * 64 + 32],
                    num_idxs=BLK, num_idxs_reg=BLK,
                    elem_size=ELEM, transpose=True,
                    queue_num=0,
                )
                nc.gpsimd.dma_gather(
                    out_ap=dstT[:].rearrange("p (c e) -> p c e", e=BLK),
                    in_ap=table_d[g * dstwin:(g + 1) * dstwin, :],
                    idxs_ap=eidx_t[:, b * 64 + 32:b * 64 + 64],
                    num_idxs=BLK, num_idxs_reg=BLK,
                    elem_size=ELEM, transpose=True,
                    queue_num=(1 if int(os.environ.get("KERNEL_NSWQ", "1")) > 1 else 0),
                )

                # pair products, already in [feat, edge] layout; chunk 7 row 0
                # is mask_src*mask_dst = bv.
                prodT = ppool.tile([128, 8 * BLK], DT, tag="prodT")
                nc.vector.tensor_tensor(
                    out=prodT[:], in0=srcT[:], in1=dstT[:], op=OP.mult)

                # first layers (contract feat chunks: 0 structural, 1..6 chem)
                p_st = phpool.tile([64, BLK], F32, tag="pst")
                nc.tensor.matmul(p_st[:], lhsT=sw1_t[:], rhs=prodT[:, 0:BLK],
                                 start=True, stop=True)
                p_cha = phpool.tile([128, BLK], F32, tag="pcha")
                for k in range(6):
                    nc.tensor.matmul(
                        p_cha[:], lhsT=cw1a_t[:, k * 128:(k + 1) * 128],
                        rhs=prodT[:, (k + 1) * BLK:(k + 2) * BLK],
                        start=(k == 0), stop=(k == 5))
                p_chb = phpool.tile([64, BLK], F32, tag="pchb")
                for k in range(6):
                    nc.tensor.matmul(
                        p_chb[:], lhsT=cw1b_t[:, k * 64:(k + 1) * 64],
                        rhs=prodT[:, (k + 1) * BLK:(k + 2) * BLK],
                        start=(k == 0), stop=(k == 5))
                p_cb = phpool.tile([128, BLK], F32, tag="pcb")
                for k in range(7):
                    nc.tensor.matmul(
                        p_cb[:], lhsT=mw1_t[:, k * 128:(k + 1) * 128],
                        rhs=prodT[:, k * BLK:(k + 1) * BLK],
                        start=(k == 0), stop=(k == 6))

                # hidden activations (relu + bias)
                hid_st = hst_bufs[b % 2]
                nc.scalar.activation(out=hid_st[0:64, :], in_=p_st[:],
                                     func=AF.Relu, bias=sb1_t[:])
                hid_cha = hpool.tile([128, BLK], DT, tag="hcha")
                nc.scalar.activation(out=hid_cha[:], in_=p_cha[:],
                                     func=AF.Relu, bias=cb1a_t[:])
                hid_chb = hpool.tile([64, BLK], DT, tag="hchb")
                nc.scalar.activation(out=hid_chb[:], in_=p_chb[:],
                                     func=AF.Relu, bias=cb1b_t[:])
                hid_cb = hpool.tile([128, BLK], DT, tag="hcb")
                nc.scalar.activation(out=hid_cb[:], in_=p_cb[:],
                                     func=AF.Relu, bias=mb1_t[:])

                # second layer: t (weighted sum incl. biases via ones-row),
                # s (structural-only score) in separate psum banks.
                p_t = pspool.tile([1, BLK], F32, tag="pt")
                nc.tensor.matmul(p_t[0:1, :], lhsT=t2st_t[:], rhs=hid_st[:],
                                 start=True, stop=False)
                nc.tensor.matmul(p_t[0:1, :], lhsT=t2cha_t[:], rhs=hid_cha[:],
                                 start=False, stop=False)
                nc.tensor.matmul(p_t[0:1, :], lhsT=t2chb_t[:], rhs=hid_chb[:],
                                 start=False, stop=False)
                nc.tensor.matmul(p_t[0:1, :], lhsT=t2cb_t[:], rhs=hid_cb[:],
                                 start=False, stop=True)
                p_s = pspool.tile([1, BLK], F32, tag="psc")
                nc.tensor.matmul(p_s[0:1, :], lhsT=s2_t[:], rhs=hid_st[:],
                                 start=True, stop=True)

                # blend: out = s + bv * (t - s); bv is prodT chunk-7 row 0.
                s_sb = bpool.tile([1, BLK], F32, tag="ssb")
                nc.scalar.activation(out=s_sb[:], in_=p_s[0:1, :], func=AF.Copy)
                d_t = bpool.tile([1, BLK], F32, tag="d")
                nc.vector.tensor_tensor(out=d_t[:], in0=p_t[0:1, :],
                                        in1=s_sb[:], op=OP.subtract)
                m_t = bpool.tile([1, BLK], F32, tag="m")
                nc.vector.tensor_tensor(out=m_t[:], in0=prodT[0:1, 7 * BLK:8 * BLK],
                                        in1=d_t[:], op=OP.mult)
                o_t = bpool.tile([1, BLK], F32, tag="o")
                nc.vector.tensor_tensor(out=o_t[:], in0=s_sb[:],
                                        in1=m_t[:], op=OP.add)
                nc.sync.dma_start(out=out_d[b:b + 1, :], in_=o_t[:])

    nc.finalize()
    return nc


def _host_prep(z, chemistry, edge, smiles_mask,
               sw1, sb1, sw2, sb2, cw1, cb1, cw2, cb2, mw1, mb1, mw2, mb2,
               path_weights, n_nodes=N_NODES, ncores=NCORES):
    """Sort/bucket edges, build the padded bf16 table + per-core shards."""
    import ml_dtypes
    wdt = ml_dtypes.bfloat16

    z = np.asarray(z, np.float32)
    chemistry = np.asarray(chemistry, np.float32)
    mask = np.asarray(smiles_mask, np.float32).reshape(-1)
    table = np.zeros((n_nodes, ELEM), np.float32)
    table[:, :SD] = z
    table[:, SD:F] = chemistry
    table[:, F] = mask
    table = table.astype(wdt)

    srcwin = min(SRCWIN, n_nodes)
    dstwin = -(-n_nodes // NBUCK)
    assert dstwin <= 32767

    pw = np.asarray(path_weights, np.float64)
    e = np.exp(pw - pw.max())
    w = e / e.sum()
    w0, w1, w2 = [float(x) for x in w]

    sw1 = np.asarray(sw1, np.float32)
    cw1 = np.asarray(cw1, np.float32)
    mw1 = np.asarray(mw1, np.float32)
    cw1a = cw1[:, :128].reshape(6, 128, 128).transpose(1, 0, 2).reshape(128, 6 * 128)
    cw1b = cw1[:, 128:].reshape(6, 128, 64).transpose(1, 0, 2).reshape(128, 6 * 64)
    mw1p = mw1.reshape(7, 128, 128).transpose(1, 0, 2).reshape(128, 7 * 128)
    b1pack = np.concatenate([
        np.asarray(sb1, np.float32),
        np.asarray(cb1, np.float32)[:128],
        np.asarray(cb1, np.float32)[128:],
        np.asarray(mb1, np.float32)]).astype(np.float32)

    sw2v = np.asarray(sw2, np.float64).reshape(-1)
    cw2v = np.asarray(cw2, np.float64).reshape(-1)
    mw2v = np.asarray(mw2, np.float64).reshape(-1)
    sb2v = float(np.asarray(sb2, np.float64).reshape(())[()])
    cb2v = float(np.asarray(cb2, np.float64).reshape(())[()])
    mb2v = float(np.asarray(mb2, np.float64).reshape(())[()])
    tb = w0 * sb2v + w1 * cb2v + w2 * mb2v
    w2pack = np.concatenate([
        np.concatenate([sw2v, [sb2v]]),
        np.concatenate([w0 * sw2v, [tb]]),
        w1 * cw2v[:128], w1 * cw2v[128:], w2 * mw2v]).astype(np.float32)
    assert w2pack.shape == (450,)

    edge = np.asarray(edge)
    E = edge.shape[0]
    src_all = edge[:, 0].astype(np.int64)
    dst_all = edge[:, 1].astype(np.int64)
    order = np.argsort(src_all, kind='stable')
    epc = E // ncores

    cores = []
    counts_all = np.zeros((ncores, NBUCK), np.int64)
    for c in range(ncores):
        ids = order[c * epc:(c + 1) * epc]
        s = src_all[ids]
        d = dst_all[ids]
        w0c = max(0, min(int(s.min()), n_nodes - srcwin))
        assert int(s.max()) - w0c < srcwin, "src window overflow"
        g = d // dstwin
        bord = np.argsort(g, kind='stable')
        ids, s, d, g = ids[bord], s[bord], d[bord], g[bord]
        counts_all[c] = np.bincount(g, minlength=NBUCK)
        cores.append((ids, s - w0c, d - g * dstwin, g, w0c))

    bucket_blocks = tuple(int(-(-int(counts_all[:, gg].max()) // BLK))
                          for gg in range(NBUCK))
    bucket_blocks = tuple(max(1, bb) for bb in bucket_blocks)
    nblk = sum(bucket_blocks)

    shards = []
    for c in range(ncores):
        ids, s_rel, d_rel, g, w0c = cores[c]
        src16 = np.zeros(nblk * BLK, np.int16)
        dst16 = np.zeros(nblk * BLK, np.int16)
        perm = np.full(nblk * BLK, -1, np.int64)
        base_blk = 0
        pos = 0
        for gg in range(NBUCK):
            n_g = int(counts_all[c, gg])
            sl = slice(base_blk * BLK, base_blk * BLK + n_g)
            src16[sl] = s_rel[pos:pos + n_g].astype(np.int16)
            dst16[sl] = d_rel[pos:pos + n_g].astype(np.int16)
            perm[sl] = ids[pos:pos + n_g]
            pos += n_g
            base_blk += bucket_blocks[gg]
        # per-block idx wrap: flat pos k -> [k%16, k//16], replicated x8
        ar = np.arange(BLK)
        eidx = np.zeros((16, nblk * 64), np.int16)
        for b in range(nblk):
            sblk = src16[b * BLK:(b + 1) * BLK]
            dblk = dst16[b * BLK:(b + 1) * BLK]
            eidx[ar % 16, b * 64 + ar // 16] = sblk
            eidx[ar % 16, b * 64 + 32 + ar // 16] = dblk
        eidx = np.tile(eidx, (8, 1))
        stable = np.ascontiguousarray(table[w0c:w0c + srcwin])
        shards.append((eidx, stable, perm))

    shared = dict(table=table, sw1=sw1.astype(wdt),
                  cw1a=np.ascontiguousarray(cw1a).astype(wdt),
                  cw1b=np.ascontiguousarray(cw1b).astype(wdt),
                  mw1p=np.ascontiguousarray(mw1p).astype(wdt),
                  b1pack=b1pack, w2pack=w2pack.astype(wdt))
    return shared, shards, bucket_blocks, srcwin, E


_BUILD_CACHE = {}


def kernel(z, chemistry, edge, smiles_mask,
           sw1, sb1, sw2, sb2, cw1, cb1, cw2, cb2, mw1, mb1, mw2, mb2,
           path_weights):
    global LAST_EXEC_NS
    from concourse import bass_utils
    from concourse.bass_utils import run_bass_kernel_spmd

    trace = os.environ.get("KERNEL_TRACE", "0") == "1"
    if trace:
        # No artifact bucket in this container; keep the NTFF trace local.
        bass_utils.upload_artifacts = lambda tmpdir: tmpdir

    shared, shards, bucket_blocks, srcwin, E = _host_prep(
        z, chemistry, edge, smiles_mask, sw1, sb1, sw2, sb2,
        cw1, cb1, cw2, cb2, mw1, mb1, mw2, mb2, path_weights)

    key = (N_NODES, bucket_blocks, srcwin)
    if key not in _BUILD_CACHE:
        _BUILD_CACHE[key] = _build(N_NODES, bucket_blocks, srcwin)
    nc = _BUILD_CACHE[key]

    in_maps = []
    for c in range(NCORES):
        m = dict(shared)
        m["eidx"], m["stable"], _ = shards[c]
        in_maps.append(m)

    tmpdir = os.environ.get("KERNEL_TRACE_DIR") or None
    res = run_bass_kernel_spmd(nc, in_maps, core_ids=list(range(NCORES)),
                               trace=trace, tmpdir=tmpdir)
    if trace:
        LAST_EXEC_NS = res.exec_time_ns

    result = np.zeros(E, np.float32)
    for c in range(NCORES):
        perm = shards[c][2]
        dev = res.results[c]["out"].reshape(-1)
        valid = perm >= 0
        result[perm[valid]] = dev[valid]
    return result
